# revision 32
# baseline (speedup 1.0000x reference)
"""AttentionBlock (GroupNorm + 4-head self-attention + proj + residual) on 8 TRN2 cores.

Sharding: core = 2*b + hh  (b = batch 0..3, hh = head-half 0..1).
Each core handles one batch image and 2 of the 4 heads (tensor-parallel over
heads for qkv/attention/proj).  GroupNorm (cheap) is recomputed on both cores
of a batch.  Each core emits a partial proj output [256, 4096]; the host sums
the two head-half partials, adds proj bias and the residual x.
"""

import sys

sys.path.insert(0, "/opt/trn_rl_repo")

import numpy as np  # noqa: E402

import concourse.bacc as bacc  # noqa: E402
import concourse.tile as tile  # noqa: E402
from concourse import mybir  # noqa: E402
from concourse.bass_utils import run_bass_kernel_spmd  # noqa: E402
from concourse.masks import make_identity  # noqa: E402

F32 = mybir.dt.float32
F32R = mybir.dt.float32r
BF16 = mybir.dt.bfloat16
FP8 = mybir.dt.float8e4
AF = mybir.ActivationFunctionType
ALU = mybir.AluOpType

# Problem constants (hardcoded per contract)
B, C, H, W = 4, 256, 64, 64
N = H * W          # 4096 pixels
NH, HD = 4, 64     # heads, head dim
GROUPS = 8
EPS = 1e-5
SCALE = HD ** -0.5  # 0.125

NCHUNK = 512            # pixel chunk (matmul moving dim)
NCH = N // NCHUNK       # 8
MCH = N // 128          # 32 m-chunks of 128 pixels
EXPG = 2                # m-chunks exp'd per ACT instruction


def build_bass():
    nc = bacc.Bacc("TRN2", target_bir_lowering=False, debug=False)

    # ---- DRAM I/O (per-core shards fed via in_maps) ----
    xd = nc.dram_tensor("x", [C, N], F32, kind="ExternalInput")
    wqkvT_d = nc.dram_tensor("wqkvT", [C, 384], F32R, kind="ExternalInput")
    qkvb_d = nc.dram_tensor("qkvb", [3, 128, 1], F32, kind="ExternalInput")
    pwT_d = nc.dram_tensor("pwT", [128, C], F32R, kind="ExternalInput")
    nw_d = nc.dram_tensor("nw", [2, 128, 1], F32, kind="ExternalInput")
    nb_d = nc.dram_tensor("nb", [2, 128, 1], F32, kind="ExternalInput")
    indf_d = nc.dram_tensor("indf", [2, 128, 8], F32, kind="ExternalInput")
    indb_d = nc.dram_tensor("indb", [2, 8, 128], F32, kind="ExternalInput")
    out_d = nc.dram_tensor("out_part", [C, N], F32, kind="ExternalOutput")

    with tile.TileContext(nc) as tc:
        with (
            tc.tile_pool(name="persist", bufs=1) as pp,
            tc.tile_pool(name="tmp", bufs=3) as tp,
            tc.tile_pool(name="small", bufs=4) as sp,
            tc.tile_pool(name="apool", bufs=3) as ap_pool,
            tc.tile_pool(name="ps_s", bufs=2, space="PSUM") as ps_s,
            tc.tile_pool(name="ps_o", bufs=2, space="PSUM") as ps_o,
        ):
            # ================= Phase 0: loads & constants =================
            x_t = []
            for i in range(2):
                xt = pp.tile([128, N], F32, tag=f"x{i}", name=f"x{i}")
                for c4 in range(4):
                    nc.sync.dma_start(
                        out=xt[:, 1024 * c4:1024 * (c4 + 1)],
                        in_=xd[128 * i:128 * (i + 1), 1024 * c4:1024 * (c4 + 1)])
                x_t.append(xt)

            wqkvT_t = []
            for i in range(2):
                wt = pp.tile([128, 384], F32R, tag=f"wqkv{i}", name=f"wqkv{i}")
                nc.sync.dma_start(out=wt, in_=wqkvT_d[128 * i:128 * (i + 1), :])
                wqkvT_t.append(wt)

            qkvb_t = []
            for j in range(3):
                bt = sp.tile([128, 1], F32, tag=f"qkvb{j}", name=f"qkvb{j}")
                nc.sync.dma_start(out=bt, in_=qkvb_d[j])
                qkvb_t.append(bt)

            pwT_t = []
            for i in range(2):
                pt = pp.tile([128, 128], F32R, tag=f"pw{i}", name=f"pw{i}")
                nc.sync.dma_start(out=pt, in_=pwT_d[:, 128 * i:128 * (i + 1)])
                pwT_t.append(pt)

            nw_t, nb_t, indf_t, indb_t = [], [], [], []
            for i in range(2):
                t1 = sp.tile([128, 1], F32, tag=f"nw{i}", name=f"nw{i}")
                nc.sync.dma_start(out=t1, in_=nw_d[i])
                nw_t.append(t1)
                t2 = sp.tile([128, 1], F32, tag=f"nb{i}", name=f"nb{i}")
                nc.sync.dma_start(out=t2, in_=nb_d[i])
                nb_t.append(t2)
                t3 = sp.tile([128, 8], F32, tag=f"indf{i}", name=f"indf{i}")
                nc.sync.dma_start(out=t3, in_=indf_d[i])
                indf_t.append(t3)
                t4 = sp.tile([8, 128], F32, tag=f"indb{i}", name=f"indb{i}")
                nc.sync.dma_start(out=t4, in_=indb_d[i])
                indb_t.append(t4)

            ident = pp.tile([128, 128], BF16, tag="ident", name="ident")
            make_identity(nc, ident)

            eps8 = sp.tile([8, 1], F32, tag="eps8", name="eps8")
            nc.vector.memset(eps8, EPS)

            # ================= Phase 1: GroupNorm =================
            SDIM = nc.vector.BN_STATS_DIM   # 6
            ADIM = nc.vector.BN_AGGR_DIM    # 2
            NSUB = N // nc.vector.BN_STATS_FMAX if N > nc.vector.BN_STATS_FMAX else 1
            SUBLEN = N // NSUB

            m1e2 = []
            for i in range(2):
                st = tp.tile([128, NSUB, SDIM], F32, tag="bnst", name=f"bnst{i}")
                for s in range(NSUB):
                    nc.vector.bn_stats(
                        out=st[:, s, :],
                        in_=x_t[i][:, SUBLEN * s:SUBLEN * (s + 1)],
                    )
                mv = tp.tile([128, ADIM], F32, tag="bnmv", name=f"bnmv{i}")
                nc.vector.bn_aggr(out=mv, in_=st)
                # build [mean, E[x^2]] = [mean, var + mean^2]
                me = sp.tile([128, 2], F32, tag=f"m1e2_{i}", name=f"m1e2_{i}")
                msq = tp.tile([128, 1], F32, tag="msq", name=f"msq{i}")
                nc.vector.tensor_mul(out=msq, in0=mv[:, 0:1], in1=mv[:, 0:1])
                nc.vector.tensor_copy(out=me[:, 0:1], in_=mv[:, 0:1])
                nc.vector.tensor_add(out=me[:, 1:2], in0=mv[:, 1:2], in1=msq)
                m1e2.append(me)

            # group sums: psum[8, 2] = sum_c ind[c, g] * [mean_c, e2_c]
            psg = ps_s.tile([8, 2], F32, tag="s", name="psg")
            nc.tensor.matmul(psg, lhsT=indf_t[0], rhs=m1e2[0], start=True, stop=False)
            nc.tensor.matmul(psg, lhsT=indf_t[1], rhs=m1e2[1], start=False, stop=True)

            sg = sp.tile([8, 2], F32, tag="sg", name="sg")
            nc.scalar.mul(out=sg, in_=psg, mul=1.0 / 32.0)  # [mean_g, e2_g]
            vg = sp.tile([8, 1], F32, tag="vg", name="vg")
            nc.vector.tensor_mul(out=vg, in0=sg[:, 0:1], in1=sg[:, 0:1])
            nc.vector.tensor_sub(out=vg, in0=sg[:, 1:2], in1=vg)  # var_g
            nc.scalar.activation(out=vg, in_=vg, func=AF.Sqrt, bias=eps8)
            nc.vector.reciprocal(out=sg[:, 1:2], in_=vg)          # rstd_g into col 1

            h_t = []
            for i in range(2):
                psc = ps_s.tile([128, 2], F32, tag="s", name=f"psc{i}")
                nc.tensor.matmul(psc, lhsT=indb_t[i], rhs=sg, start=True, stop=True)
                sc = sp.tile([128, 1], F32, tag=f"sc{i}", name=f"sc{i}")
                off = sp.tile([128, 1], F32, tag=f"off{i}", name=f"off{i}")
                nc.vector.tensor_mul(out=sc, in0=psc[:, 1:2], in1=nw_t[i])
                nc.vector.tensor_mul(out=off, in0=psc[:, 0:1], in1=sc)
                nc.vector.tensor_sub(out=off, in0=nb_t[i], in1=off)
                ht = pp.tile([128, N], F32R, tag=f"h{i}", name=f"h{i}")
                nc.vector.tensor_scalar(
                    out=ht, in0=x_t[i], scalar1=sc, scalar2=off,
                    op0=ALU.mult, op1=ALU.add,
                )
                h_t.append(ht)

            # ================= Phase 2: qkv (o-layout) =================
            qT = pp.tile([128, N], FP8, tag="qT", name="qT")
            kT = pp.tile([128, N], FP8, tag="kT", name="kT")
            vT = pp.tile([128, N], BF16, tag="vT", name="vT")
            dests = [qT, kT, vT]
            for oi in range(3):
                for n in range(NCH):
                    ps = ps_s.tile([128, NCHUNK], F32, tag="s", name=f"qkv{oi}_{n}")
                    for ci in range(2):
                        nc.tensor.matmul(
                            ps,
                            lhsT=wqkvT_t[ci][:, 128 * oi:128 * (oi + 1)],
                            rhs=h_t[ci][:, NCHUNK * n:NCHUNK * (n + 1)],
                            start=(ci == 0), stop=(ci == 1),
                        )
                    nc.vector.tensor_scalar(
                        out=dests[oi][:, NCHUNK * n:NCHUNK * (n + 1)],
                        in0=ps, scalar1=qkvb_t[oi], scalar2=None, op0=ALU.add,
                    )

            # v into [pixel, d] layout: PE transpose of vT 128x128 tiles.
            # Per head: v_h[:, g, pair, :] = [v(64) | ones]; DoubleRow lhsT
            # slices [128, 2, 65] are contiguous; sums land at out row 64.
            v_all = pp.tile([128, MCH, 130], BF16, tag="v_all", name="v_all")
            nc.gpsimd.memset(v_all[:, :, 64:65], 1.0)
            nc.gpsimd.memset(v_all[:, :, 129:130], 1.0)
            for j in range(MCH):
                pst = ps_s.tile([128, 128], BF16, tag="s", name=f"vtr{j}")
                nc.tensor.transpose(pst, vT[:, 128 * j:128 * (j + 1)], ident)
                nc.vector.tensor_copy(out=v_all[:, j, 0:64], in_=pst[:, 0:64])
                nc.vector.tensor_copy(out=v_all[:, j, 65:129], in_=pst[:, 64:128])

            # fp8-packed q/k for DoubleRow scores: [64, N] -> [32, 2, N] per head
            q_pk, k_pk = [], []
            for hh in range(2):
                qp = pp.tile([32, 2, N], FP8, tag=f"qpk{hh}", name=f"qpk{hh}")
                nc.sync.dma_start(out=qp, in_=qT[64 * hh:64 * (hh + 1), :])
                q_pk.append(qp)
                kp = pp.tile([32, 2, N], FP8, tag=f"kpk{hh}", name=f"kpk{hh}")
                nc.sync.dma_start(out=kp, in_=kT[64 * hh:64 * (hh + 1), :])
                k_pk.append(kp)

            # ================= Phase 3: attention + proj =================
            # Software-pipelined: chunk n's normalization+proj tail is emitted
            # in the middle of chunk n+1's score/attnv group loop so the PE
            # never waits on the DVE/GPSIMD reciprocal chain.
            def emit_tail(po, n):
                nsl = slice(NCHUNK * n, NCHUNK * (n + 1))
                rbh = []
                for hh in range(2):
                    rr = tp.tile([1, NCHUNK], F32, tag=f"rr{hh}", name=f"rr{hh}_{n}")
                    nc.vector.tensor_copy(out=rr, in_=po[hh][64:65, :])
                    nc.vector.reciprocal_approx_fast(out=rr, in_=rr)
                    rb = tp.tile([64, NCHUNK], F32, tag=f"rb{hh}", name=f"rb{hh}_{n}")
                    nc.gpsimd.partition_broadcast(rb, rr, channels=64)
                    rbh.append(rb)
                onrm = tp.tile([128, NCHUNK], F32R, tag="onrm", name=f"onrm_{n}")
                nc.vector.tensor_mul(out=onrm[0:64, :], in0=po[0][0:64, :], in1=rbh[0])
                nc.vector.tensor_mul(out=onrm[64:128, :], in0=po[1][0:64, :], in1=rbh[1])
                for ci in range(2):
                    ppj = ps_s.tile([128, NCHUNK], F32, tag="s", name=f"proj{ci}_{n}")
                    nc.tensor.matmul(ppj, lhsT=pwT_t[ci], rhs=onrm, start=True, stop=True)
                    osb = tp.tile([128, NCHUNK], F32, tag="osb", name=f"osb{ci}_{n}")
                    nc.vector.tensor_copy(out=osb, in_=ppj)
                    nc.sync.dma_start(out=out_d[128 * ci:128 * (ci + 1), nsl], in_=osb)

            pending = None
            for n in range(NCH):
                nsl = slice(NCHUNK * n, NCHUNK * (n + 1))
                po = []
                for hh in range(2):
                    poh = ps_o.tile([65, NCHUNK], F32, tag=f"o{hh}", name=f"po{hh}_{n}")
                    dsl = slice(64 * hh, 64 * (hh + 1))
                    for g in range(MCH // EXPG):
                        pss = ps_s.tile([128, EXPG, NCHUNK], F32, tag="s", name=f"s{n}_{hh}_{g}")
                        for u in range(EXPG):
                            j = EXPG * g + u
                            nc.tensor.matmul(
                                pss[:, u, :],
                                lhsT=k_pk[hh][:, :, 128 * j:128 * (j + 1)],
                                rhs=q_pk[hh][:, :, nsl],
                                perf_mode=mybir.MatmulPerfMode.DoubleRow,
                                start=True, stop=True,
                            )
                        at = ap_pool.tile([128, EXPG, NCHUNK], BF16, tag="a", name=f"a{n}_{hh}_{g}")
                        nc.scalar.activation(out=at, in_=pss, func=AF.Exp, scale=SCALE)
                        for u in range(EXPG):
                            j = EXPG * g + u
                            lhs = v_all[:, j, 0:65] if hh == 0 else v_all[:, j, 65:130]
                            nc.tensor.matmul(
                                poh, lhsT=lhs, rhs=at[:, u, :],
                                start=(j == 0), stop=(j == MCH - 1),
                            )
                        if pending is not None and hh == 0 and g == 3:
                            emit_tail(*pending)
                            pending = None
                    po.append(poh)
                pending = (po, n)
            emit_tail(*pending)

    nc.compile()
    return nc


_NC_CACHE = None


def _get_nc():
    global _NC_CACHE
    if _NC_CACHE is None:
        _NC_CACHE = build_bass()
    return _NC_CACHE


def _make_in_maps(x, norm_w, norm_b, qkv_w, qkv_b, proj_w):
    # constant index helper tensors
    ch = np.arange(128)
    indf = np.zeros((2, 128, 8), np.float32)
    indb = np.zeros((2, 8, 128), np.float32)
    for i in range(2):
        g = (i * 128 + ch) // 32
        indf[i, ch, g] = 1.0
        indb[i, g, ch] = 1.0
    nw = norm_w.reshape(2, 128, 1).astype(np.float32)
    nb = norm_b.reshape(2, 128, 1).astype(np.float32)

    in_maps = []
    for core in range(8):
        b, hh = core // 2, core % 2
        sl = slice(128 * hh, 128 * (hh + 1))
        w_slice = np.concatenate(
            [qkv_w[sl], qkv_w[256 + 128 * hh:256 + 128 * (hh + 1)],
             qkv_w[512 + 128 * hh:512 + 128 * (hh + 1)]], axis=0,
        )  # [384, 256]
        wqkvT = np.ascontiguousarray(w_slice.T).astype(np.float32)  # [256, 384]
        qkvb = np.stack(
            [qkv_b[sl], qkv_b[256 + 128 * hh:256 + 128 * (hh + 1)],
             qkv_b[512 + 128 * hh:512 + 128 * (hh + 1)]], axis=0,
        ).reshape(3, 128, 1).astype(np.float32)
        pwT = np.ascontiguousarray(proj_w[:, sl].T).astype(np.float32)  # [128, 256]
        in_maps.append({
            "x": np.ascontiguousarray(x[b].reshape(C, N)).astype(np.float32),
            "wqkvT": wqkvT,
            "qkvb": qkvb,
            "pwT": pwT,
            "nw": nw,
            "nb": nb,
            "indf": indf,
            "indb": indb,
        })
    return in_maps


def kernel(x, norm_w, norm_b, qkv_w, qkv_b, proj_w, proj_b, _trace=False, _tmpdir=None):
    x = np.asarray(x, np.float32)
    norm_w = np.asarray(norm_w, np.float32)
    norm_b = np.asarray(norm_b, np.float32)
    qkv_w = np.asarray(qkv_w, np.float32)
    qkv_b = np.asarray(qkv_b, np.float32)
    proj_w = np.asarray(proj_w, np.float32)
    proj_b = np.asarray(proj_b, np.float32)

    nc = _get_nc()
    in_maps = _make_in_maps(x, norm_w, norm_b, qkv_w, qkv_b, proj_w)
    kw = {}
    if _trace:
        kw = dict(trace=True, tmpdir=_tmpdir)
    res = run_bass_kernel_spmd(nc, in_maps, list(range(8)), **kw)

    out = np.empty((B, C, H, W), np.float32)
    bias_res = proj_b[:, None].astype(np.float32)
    for b in range(B):
        acc = (res.results[2 * b]["out_part"] + res.results[2 * b + 1]["out_part"]
               + bias_res + x[b].reshape(C, N))
        out[b] = acc.reshape(C, H, W)
    if _trace:
        return out, res
    return out


# revision 33
# speedup vs baseline: 1.4234x; 1.4234x over previous
"""AttentionBlock (GroupNorm + 4-head self-attention + proj + residual) on 8 TRN2 cores.

Sharding: core = 2*b + hh  (b = batch 0..3, hh = head-half 0..1).
Each core handles one batch image and 2 of the 4 heads (tensor-parallel over
heads for qkv/attention/proj).  GroupNorm (cheap) is recomputed on both cores
of a batch.  Each core emits a partial proj output [256, 4096]; the host sums
the two head-half partials, adds proj bias and the residual x.
"""

import sys

sys.path.insert(0, "/opt/trn_rl_repo")

import numpy as np  # noqa: E402

import concourse.bacc as bacc  # noqa: E402
import concourse.tile as tile  # noqa: E402
from concourse import mybir  # noqa: E402
from concourse.bass_utils import run_bass_kernel_spmd  # noqa: E402
from concourse.masks import make_identity  # noqa: E402

F32 = mybir.dt.float32
F32R = mybir.dt.float32r
BF16 = mybir.dt.bfloat16
FP8 = mybir.dt.float8e4
AF = mybir.ActivationFunctionType
ALU = mybir.AluOpType

# Problem constants (hardcoded per contract)
B, C, H, W = 4, 256, 64, 64
N = H * W          # 4096 pixels
NH, HD = 4, 64     # heads, head dim
GROUPS = 8
EPS = 1e-5
SCALE = HD ** -0.5  # 0.125

NCHUNK = 512            # pixel chunk (matmul moving dim)
NCH = N // NCHUNK       # 8
MCH = N // 128          # 32 m-chunks of 128 pixels
EXPG = 2                # m-chunks exp'd per ACT instruction


def build_bass():
    nc = bacc.Bacc("TRN2", target_bir_lowering=False, debug=False)

    # ---- DRAM I/O (per-core shards fed via in_maps) ----
    xd = nc.dram_tensor("x", [C, N], F32, kind="ExternalInput")
    wqkvT_d = nc.dram_tensor("wqkvT", [C, 384], F32R, kind="ExternalInput")
    qkvb_d = nc.dram_tensor("qkvb", [3, 128, 1], F32, kind="ExternalInput")
    pwT_d = nc.dram_tensor("pwT", [128, C], F32R, kind="ExternalInput")
    nw_d = nc.dram_tensor("nw", [2, 128, 1], F32, kind="ExternalInput")
    nb_d = nc.dram_tensor("nb", [2, 128, 1], F32, kind="ExternalInput")
    indf_d = nc.dram_tensor("indf", [2, 128, 8], F32, kind="ExternalInput")
    indb_d = nc.dram_tensor("indb", [2, 8, 128], F32, kind="ExternalInput")
    out_d = nc.dram_tensor("out_part", [C, N], F32, kind="ExternalOutput")

    with tile.TileContext(nc) as tc:
        with (
            tc.tile_pool(name="persist", bufs=1) as pp,
            tc.tile_pool(name="tmp", bufs=3) as tp,
            tc.tile_pool(name="small", bufs=4) as sp,
            tc.tile_pool(name="apool", bufs=3) as ap_pool,
            tc.tile_pool(name="ps_s", bufs=2, space="PSUM") as ps_s,
            tc.tile_pool(name="ps_o", bufs=2, space="PSUM") as ps_o,
        ):
            # ================= Phase 0: loads & constants =================
            x_t = []
            for i in range(2):
                xt = pp.tile([128, N], F32, tag=f"x{i}", name=f"x{i}")
                for c4 in range(4):
                    nc.sync.dma_start(
                        out=xt[:, 1024 * c4:1024 * (c4 + 1)],
                        in_=xd[128 * i:128 * (i + 1), 1024 * c4:1024 * (c4 + 1)])
                x_t.append(xt)

            wqkvT_t = []
            for i in range(2):
                wt = pp.tile([128, 384], F32R, tag=f"wqkv{i}", name=f"wqkv{i}")
                nc.sync.dma_start(out=wt, in_=wqkvT_d[128 * i:128 * (i + 1), :])
                wqkvT_t.append(wt)

            qkvb_t = []
            for j in range(3):
                bt = sp.tile([128, 1], F32, tag=f"qkvb{j}", name=f"qkvb{j}")
                nc.sync.dma_start(out=bt, in_=qkvb_d[j])
                qkvb_t.append(bt)

            pwT_t = []
            for i in range(2):
                pt = pp.tile([128, 128], F32R, tag=f"pw{i}", name=f"pw{i}")
                nc.sync.dma_start(out=pt, in_=pwT_d[:, 128 * i:128 * (i + 1)])
                pwT_t.append(pt)

            nw_t, nb_t, indf_t, indb_t = [], [], [], []
            for i in range(2):
                t1 = sp.tile([128, 1], F32, tag=f"nw{i}", name=f"nw{i}")
                nc.sync.dma_start(out=t1, in_=nw_d[i])
                nw_t.append(t1)
                t2 = sp.tile([128, 1], F32, tag=f"nb{i}", name=f"nb{i}")
                nc.sync.dma_start(out=t2, in_=nb_d[i])
                nb_t.append(t2)
                t3 = sp.tile([128, 8], F32, tag=f"indf{i}", name=f"indf{i}")
                nc.sync.dma_start(out=t3, in_=indf_d[i])
                indf_t.append(t3)
                t4 = sp.tile([8, 128], F32, tag=f"indb{i}", name=f"indb{i}")
                nc.sync.dma_start(out=t4, in_=indb_d[i])
                indb_t.append(t4)

            ident = pp.tile([128, 128], BF16, tag="ident", name="ident")
            make_identity(nc, ident)

            eps8 = sp.tile([8, 1], F32, tag="eps8", name="eps8")
            nc.vector.memset(eps8, EPS)

            # ================= Phase 1: GroupNorm =================
            SDIM = nc.vector.BN_STATS_DIM   # 6
            ADIM = nc.vector.BN_AGGR_DIM    # 2
            NSUB = N // nc.vector.BN_STATS_FMAX if N > nc.vector.BN_STATS_FMAX else 1
            SUBLEN = N // NSUB

            m1e2 = []
            for i in range(2):
                st = tp.tile([128, NSUB, SDIM], F32, tag="bnst", name=f"bnst{i}")
                for s in range(NSUB):
                    nc.vector.bn_stats(
                        out=st[:, s, :],
                        in_=x_t[i][:, SUBLEN * s:SUBLEN * (s + 1)],
                    )
                mv = tp.tile([128, ADIM], F32, tag="bnmv", name=f"bnmv{i}")
                nc.vector.bn_aggr(out=mv, in_=st)
                # build [mean, E[x^2]] = [mean, var + mean^2]
                me = sp.tile([128, 2], F32, tag=f"m1e2_{i}", name=f"m1e2_{i}")
                msq = tp.tile([128, 1], F32, tag="msq", name=f"msq{i}")
                nc.vector.tensor_mul(out=msq, in0=mv[:, 0:1], in1=mv[:, 0:1])
                nc.vector.tensor_copy(out=me[:, 0:1], in_=mv[:, 0:1])
                nc.vector.tensor_add(out=me[:, 1:2], in0=mv[:, 1:2], in1=msq)
                m1e2.append(me)

            # group sums: psum[8, 2] = sum_c ind[c, g] * [mean_c, e2_c]
            psg = ps_s.tile([8, 2], F32, tag="s", name="psg")
            nc.tensor.matmul(psg, lhsT=indf_t[0], rhs=m1e2[0], start=True, stop=False)
            nc.tensor.matmul(psg, lhsT=indf_t[1], rhs=m1e2[1], start=False, stop=True)

            sg = sp.tile([8, 2], F32, tag="sg", name="sg")
            nc.scalar.mul(out=sg, in_=psg, mul=1.0 / 32.0)  # [mean_g, e2_g]
            vg = sp.tile([8, 1], F32, tag="vg", name="vg")
            nc.vector.tensor_mul(out=vg, in0=sg[:, 0:1], in1=sg[:, 0:1])
            nc.vector.tensor_sub(out=vg, in0=sg[:, 1:2], in1=vg)  # var_g
            nc.scalar.activation(out=vg, in_=vg, func=AF.Sqrt, bias=eps8)
            nc.vector.reciprocal(out=sg[:, 1:2], in_=vg)          # rstd_g into col 1

            h_t = []
            for i in range(2):
                psc = ps_s.tile([128, 2], F32, tag="s", name=f"psc{i}")
                nc.tensor.matmul(psc, lhsT=indb_t[i], rhs=sg, start=True, stop=True)
                sc = sp.tile([128, 1], F32, tag=f"sc{i}", name=f"sc{i}")
                off = sp.tile([128, 1], F32, tag=f"off{i}", name=f"off{i}")
                nc.vector.tensor_mul(out=sc, in0=psc[:, 1:2], in1=nw_t[i])
                nc.vector.tensor_mul(out=off, in0=psc[:, 0:1], in1=sc)
                nc.vector.tensor_sub(out=off, in0=nb_t[i], in1=off)
                ht = pp.tile([128, N], F32R, tag=f"h{i}", name=f"h{i}")
                nc.vector.tensor_scalar(
                    out=ht, in0=x_t[i], scalar1=sc, scalar2=off,
                    op0=ALU.mult, op1=ALU.add,
                )
                h_t.append(ht)

            # ================= Phase 2: qkv (o-layout) =================
            qT = pp.tile([128, N], BF16, tag="qT", name="qT")
            kT = pp.tile([128, N], BF16, tag="kT", name="kT")
            vT = pp.tile([128, N], BF16, tag="vT", name="vT")
            dests = [qT, kT, vT]
            for oi in range(3):
                for n in range(NCH):
                    ps = ps_s.tile([128, NCHUNK], F32, tag="s", name=f"qkv{oi}_{n}")
                    for ci in range(2):
                        nc.tensor.matmul(
                            ps,
                            lhsT=wqkvT_t[ci][:, 128 * oi:128 * (oi + 1)],
                            rhs=h_t[ci][:, NCHUNK * n:NCHUNK * (n + 1)],
                            start=(ci == 0), stop=(ci == 1),
                        )
                    nc.vector.tensor_scalar(
                        out=dests[oi][:, NCHUNK * n:NCHUNK * (n + 1)],
                        in0=ps, scalar1=qkvb_t[oi], scalar2=None, op0=ALU.add,
                    )

            # v into [pixel, d] layout: PE transpose of vT 128x128 tiles.
            # Per head: v_h[:, g, pair, :] = [v(64) | ones]; DoubleRow lhsT
            # slices [128, 2, 65] are contiguous; sums land at out row 64.
            v_all = pp.tile([128, MCH, 130], BF16, tag="v_all", name="v_all")
            nc.gpsimd.memset(v_all[:, :, 64:65], 1.0)
            nc.gpsimd.memset(v_all[:, :, 129:130], 1.0)
            for j in range(MCH):
                pst = ps_s.tile([128, 128], BF16, tag="s", name=f"vtr{j}")
                nc.tensor.transpose(pst, vT[:, 128 * j:128 * (j + 1)], ident)
                nc.vector.tensor_copy(out=v_all[:, j, 0:64], in_=pst[:, 0:64])
                nc.vector.tensor_copy(out=v_all[:, j, 65:129], in_=pst[:, 64:128])


            # ================= Phase 3: attention + proj =================
            # Software-pipelined: chunk n's normalization+proj tail is emitted
            # in the middle of chunk n+1's score/attnv group loop so the PE
            # never waits on the DVE/GPSIMD reciprocal chain.
            def emit_tail(po, n):
                nsl = slice(NCHUNK * n, NCHUNK * (n + 1))
                rbh = []
                for hh in range(2):
                    rr = tp.tile([1, NCHUNK], F32, tag=f"rr{hh}", name=f"rr{hh}_{n}")
                    nc.vector.tensor_copy(out=rr, in_=po[hh][64:65, :])
                    nc.vector.reciprocal_approx_fast(out=rr, in_=rr)
                    rb = tp.tile([64, NCHUNK], F32, tag=f"rb{hh}", name=f"rb{hh}_{n}")
                    nc.gpsimd.partition_broadcast(rb, rr, channels=64)
                    rbh.append(rb)
                onrm = tp.tile([128, NCHUNK], F32R, tag="onrm", name=f"onrm_{n}")
                nc.vector.tensor_mul(out=onrm[0:64, :], in0=po[0][0:64, :], in1=rbh[0])
                nc.vector.tensor_mul(out=onrm[64:128, :], in0=po[1][0:64, :], in1=rbh[1])
                for ci in range(2):
                    ppj = ps_s.tile([128, NCHUNK], F32, tag="s", name=f"proj{ci}_{n}")
                    nc.tensor.matmul(ppj, lhsT=pwT_t[ci], rhs=onrm, start=True, stop=True)
                    osb = tp.tile([128, NCHUNK], F32, tag="osb", name=f"osb{ci}_{n}")
                    nc.vector.tensor_copy(out=osb, in_=ppj)
                    nc.sync.dma_start(out=out_d[128 * ci:128 * (ci + 1), nsl], in_=osb)

            pending = None
            for n in range(NCH):
                nsl = slice(NCHUNK * n, NCHUNK * (n + 1))
                po = []
                for hh in range(2):
                    poh = ps_o.tile([65, NCHUNK], F32, tag=f"o{hh}", name=f"po{hh}_{n}")
                    dsl = slice(64 * hh, 64 * (hh + 1))
                    for g in range(MCH // EXPG):
                        pss = ps_s.tile([128, EXPG, NCHUNK], F32, tag="s", name=f"s{n}_{hh}_{g}")
                        for u in range(EXPG):
                            j = EXPG * g + u
                            nc.tensor.matmul(
                                pss[:, u, :],
                                lhsT=kT[dsl, 128 * j:128 * (j + 1)],
                                rhs=qT[dsl, nsl],
                                start=True, stop=True,
                            )
                        at = ap_pool.tile([128, EXPG, NCHUNK], BF16, tag="a", name=f"a{n}_{hh}_{g}")
                        nc.scalar.activation(out=at, in_=pss, func=AF.Exp, scale=SCALE)
                        for u in range(EXPG):
                            j = EXPG * g + u
                            lhs = v_all[:, j, 0:65] if hh == 0 else v_all[:, j, 65:130]
                            nc.tensor.matmul(
                                poh, lhsT=lhs, rhs=at[:, u, :],
                                start=(j == 0), stop=(j == MCH - 1),
                            )
                        if pending is not None and hh == 0 and g == 3:
                            emit_tail(*pending)
                            pending = None
                    po.append(poh)
                pending = (po, n)
            emit_tail(*pending)

    nc.compile()
    return nc


_NC_CACHE = None


def _get_nc():
    global _NC_CACHE
    if _NC_CACHE is None:
        _NC_CACHE = build_bass()
    return _NC_CACHE


def _make_in_maps(x, norm_w, norm_b, qkv_w, qkv_b, proj_w):
    # constant index helper tensors
    ch = np.arange(128)
    indf = np.zeros((2, 128, 8), np.float32)
    indb = np.zeros((2, 8, 128), np.float32)
    for i in range(2):
        g = (i * 128 + ch) // 32
        indf[i, ch, g] = 1.0
        indb[i, g, ch] = 1.0
    nw = norm_w.reshape(2, 128, 1).astype(np.float32)
    nb = norm_b.reshape(2, 128, 1).astype(np.float32)

    in_maps = []
    for core in range(8):
        b, hh = core // 2, core % 2
        sl = slice(128 * hh, 128 * (hh + 1))
        w_slice = np.concatenate(
            [qkv_w[sl], qkv_w[256 + 128 * hh:256 + 128 * (hh + 1)],
             qkv_w[512 + 128 * hh:512 + 128 * (hh + 1)]], axis=0,
        )  # [384, 256]
        wqkvT = np.ascontiguousarray(w_slice.T).astype(np.float32)  # [256, 384]
        qkvb = np.stack(
            [qkv_b[sl], qkv_b[256 + 128 * hh:256 + 128 * (hh + 1)],
             qkv_b[512 + 128 * hh:512 + 128 * (hh + 1)]], axis=0,
        ).reshape(3, 128, 1).astype(np.float32)
        pwT = np.ascontiguousarray(proj_w[:, sl].T).astype(np.float32)  # [128, 256]
        in_maps.append({
            "x": np.ascontiguousarray(x[b].reshape(C, N)).astype(np.float32),
            "wqkvT": wqkvT,
            "qkvb": qkvb,
            "pwT": pwT,
            "nw": nw,
            "nb": nb,
            "indf": indf,
            "indb": indb,
        })
    return in_maps


def kernel(x, norm_w, norm_b, qkv_w, qkv_b, proj_w, proj_b, _trace=False, _tmpdir=None):
    x = np.asarray(x, np.float32)
    norm_w = np.asarray(norm_w, np.float32)
    norm_b = np.asarray(norm_b, np.float32)
    qkv_w = np.asarray(qkv_w, np.float32)
    qkv_b = np.asarray(qkv_b, np.float32)
    proj_w = np.asarray(proj_w, np.float32)
    proj_b = np.asarray(proj_b, np.float32)

    nc = _get_nc()
    in_maps = _make_in_maps(x, norm_w, norm_b, qkv_w, qkv_b, proj_w)
    kw = {}
    if _trace:
        kw = dict(trace=True, tmpdir=_tmpdir)
    res = run_bass_kernel_spmd(nc, in_maps, list(range(8)), **kw)

    out = np.empty((B, C, H, W), np.float32)
    bias_res = proj_b[:, None].astype(np.float32)
    for b in range(B):
        acc = (res.results[2 * b]["out_part"] + res.results[2 * b + 1]["out_part"]
               + bias_res + x[b].reshape(C, N))
        out[b] = acc.reshape(C, H, W)
    if _trace:
        return out, res
    return out


# revision 34
# speedup vs baseline: 1.4256x; 1.0015x over previous
"""AttentionBlock (GroupNorm + 4-head self-attention + proj + residual) on 8 TRN2 cores.

Sharding: core = 2*b + hh  (b = batch 0..3, hh = head-half 0..1).
Each core handles one batch image and 2 of the 4 heads (tensor-parallel over
heads for qkv/attention/proj).  GroupNorm (cheap) is recomputed on both cores
of a batch.  Each core emits a partial proj output [256, 4096]; the host sums
the two head-half partials, adds proj bias and the residual x.
"""

import sys

sys.path.insert(0, "/opt/trn_rl_repo")

import numpy as np  # noqa: E402

import concourse.bacc as bacc  # noqa: E402
import concourse.tile as tile  # noqa: E402
from concourse import mybir  # noqa: E402
from concourse.bass_utils import run_bass_kernel_spmd  # noqa: E402
from concourse.masks import make_identity  # noqa: E402

F32 = mybir.dt.float32
F32R = mybir.dt.float32r
BF16 = mybir.dt.bfloat16
FP8 = mybir.dt.float8e4
AF = mybir.ActivationFunctionType
ALU = mybir.AluOpType

# Problem constants (hardcoded per contract)
B, C, H, W = 4, 256, 64, 64
N = H * W          # 4096 pixels
NH, HD = 4, 64     # heads, head dim
GROUPS = 8
EPS = 1e-5
SCALE = HD ** -0.5  # 0.125

NCHUNK = 512            # pixel chunk (matmul moving dim)
NCH = N // NCHUNK       # 8
MCH = N // 128          # 32 m-chunks of 128 pixels
EXPG = 2                # m-chunks exp'd per ACT instruction


def build_bass():
    nc = bacc.Bacc("TRN2", target_bir_lowering=False, debug=False)

    # ---- DRAM I/O (per-core shards fed via in_maps) ----
    xd = nc.dram_tensor("x", [C, N], F32, kind="ExternalInput")
    wqkvT_d = nc.dram_tensor("wqkvT", [C, 384], F32R, kind="ExternalInput")
    qkvb_d = nc.dram_tensor("qkvb", [3, 128, 1], F32, kind="ExternalInput")
    pwT_d = nc.dram_tensor("pwT", [128, C], F32R, kind="ExternalInput")
    nw_d = nc.dram_tensor("nw", [2, 128, 1], F32, kind="ExternalInput")
    nb_d = nc.dram_tensor("nb", [2, 128, 1], F32, kind="ExternalInput")
    indf_d = nc.dram_tensor("indf", [2, 128, 8], F32, kind="ExternalInput")
    indb_d = nc.dram_tensor("indb", [2, 8, 128], F32, kind="ExternalInput")
    out_d = nc.dram_tensor("out_part", [C, N], F32, kind="ExternalOutput")

    with tile.TileContext(nc) as tc:
        with (
            tc.tile_pool(name="persist", bufs=1) as pp,
            tc.tile_pool(name="tmp", bufs=4) as tp,
            tc.tile_pool(name="small", bufs=4) as sp,
            tc.tile_pool(name="apool", bufs=4) as ap_pool,
            tc.tile_pool(name="ps_s", bufs=2, space="PSUM") as ps_s,
            tc.tile_pool(name="ps_o", bufs=2, space="PSUM") as ps_o,
        ):
            # ================= Phase 0: loads & constants =================
            x_t = []
            for i in range(2):
                xt = pp.tile([128, N], F32, tag=f"x{i}", name=f"x{i}")
                for c4 in range(4):
                    nc.sync.dma_start(
                        out=xt[:, 1024 * c4:1024 * (c4 + 1)],
                        in_=xd[128 * i:128 * (i + 1), 1024 * c4:1024 * (c4 + 1)])
                x_t.append(xt)

            wqkvT_t = []
            for i in range(2):
                wt = pp.tile([128, 384], F32R, tag=f"wqkv{i}", name=f"wqkv{i}")
                nc.sync.dma_start(out=wt, in_=wqkvT_d[128 * i:128 * (i + 1), :])
                wqkvT_t.append(wt)

            qkvb_t = []
            for j in range(3):
                bt = sp.tile([128, 1], F32, tag=f"qkvb{j}", name=f"qkvb{j}")
                nc.sync.dma_start(out=bt, in_=qkvb_d[j])
                qkvb_t.append(bt)

            pwT_t = []
            for i in range(2):
                pt = pp.tile([128, 128], F32R, tag=f"pw{i}", name=f"pw{i}")
                nc.sync.dma_start(out=pt, in_=pwT_d[:, 128 * i:128 * (i + 1)])
                pwT_t.append(pt)

            nw_t, nb_t, indf_t, indb_t = [], [], [], []
            for i in range(2):
                t1 = sp.tile([128, 1], F32, tag=f"nw{i}", name=f"nw{i}")
                nc.sync.dma_start(out=t1, in_=nw_d[i])
                nw_t.append(t1)
                t2 = sp.tile([128, 1], F32, tag=f"nb{i}", name=f"nb{i}")
                nc.sync.dma_start(out=t2, in_=nb_d[i])
                nb_t.append(t2)
                t3 = sp.tile([128, 8], F32, tag=f"indf{i}", name=f"indf{i}")
                nc.sync.dma_start(out=t3, in_=indf_d[i])
                indf_t.append(t3)
                t4 = sp.tile([8, 128], F32, tag=f"indb{i}", name=f"indb{i}")
                nc.sync.dma_start(out=t4, in_=indb_d[i])
                indb_t.append(t4)

            ident = pp.tile([128, 128], BF16, tag="ident", name="ident")
            make_identity(nc, ident)

            eps8 = sp.tile([8, 1], F32, tag="eps8", name="eps8")
            nc.vector.memset(eps8, EPS)

            # ================= Phase 1: GroupNorm =================
            SDIM = nc.vector.BN_STATS_DIM   # 6
            ADIM = nc.vector.BN_AGGR_DIM    # 2
            NSUB = N // nc.vector.BN_STATS_FMAX if N > nc.vector.BN_STATS_FMAX else 1
            SUBLEN = N // NSUB

            m1e2 = []
            for i in range(2):
                st = tp.tile([128, NSUB, SDIM], F32, tag="bnst", name=f"bnst{i}")
                for s in range(NSUB):
                    nc.vector.bn_stats(
                        out=st[:, s, :],
                        in_=x_t[i][:, SUBLEN * s:SUBLEN * (s + 1)],
                    )
                mv = tp.tile([128, ADIM], F32, tag="bnmv", name=f"bnmv{i}")
                nc.vector.bn_aggr(out=mv, in_=st)
                # build [mean, E[x^2]] = [mean, var + mean^2]
                me = sp.tile([128, 2], F32, tag=f"m1e2_{i}", name=f"m1e2_{i}")
                msq = tp.tile([128, 1], F32, tag="msq", name=f"msq{i}")
                nc.vector.tensor_mul(out=msq, in0=mv[:, 0:1], in1=mv[:, 0:1])
                nc.vector.tensor_copy(out=me[:, 0:1], in_=mv[:, 0:1])
                nc.vector.tensor_add(out=me[:, 1:2], in0=mv[:, 1:2], in1=msq)
                m1e2.append(me)

            # group sums: psum[8, 2] = sum_c ind[c, g] * [mean_c, e2_c]
            psg = ps_s.tile([8, 2], F32, tag="s", name="psg")
            nc.tensor.matmul(psg, lhsT=indf_t[0], rhs=m1e2[0], start=True, stop=False)
            nc.tensor.matmul(psg, lhsT=indf_t[1], rhs=m1e2[1], start=False, stop=True)

            sg = sp.tile([8, 2], F32, tag="sg", name="sg")
            nc.scalar.mul(out=sg, in_=psg, mul=1.0 / 32.0)  # [mean_g, e2_g]
            vg = sp.tile([8, 1], F32, tag="vg", name="vg")
            nc.vector.tensor_mul(out=vg, in0=sg[:, 0:1], in1=sg[:, 0:1])
            nc.vector.tensor_sub(out=vg, in0=sg[:, 1:2], in1=vg)  # var_g
            nc.scalar.activation(out=vg, in_=vg, func=AF.Sqrt, bias=eps8)
            nc.vector.reciprocal(out=sg[:, 1:2], in_=vg)          # rstd_g into col 1

            h_t = []
            for i in range(2):
                psc = ps_s.tile([128, 2], F32, tag="s", name=f"psc{i}")
                nc.tensor.matmul(psc, lhsT=indb_t[i], rhs=sg, start=True, stop=True)
                sc = sp.tile([128, 1], F32, tag=f"sc{i}", name=f"sc{i}")
                off = sp.tile([128, 1], F32, tag=f"off{i}", name=f"off{i}")
                nc.vector.tensor_mul(out=sc, in0=psc[:, 1:2], in1=nw_t[i])
                nc.vector.tensor_mul(out=off, in0=psc[:, 0:1], in1=sc)
                nc.vector.tensor_sub(out=off, in0=nb_t[i], in1=off)
                ht = pp.tile([128, N], F32R, tag=f"h{i}", name=f"h{i}")
                nc.vector.tensor_scalar(
                    out=ht, in0=x_t[i], scalar1=sc, scalar2=off,
                    op0=ALU.mult, op1=ALU.add,
                )
                h_t.append(ht)

            # ================= Phase 2: qkv (o-layout) =================
            qT = pp.tile([128, N], BF16, tag="qT", name="qT")
            kT = pp.tile([128, N], BF16, tag="kT", name="kT")
            vT = pp.tile([128, N], BF16, tag="vT", name="vT")
            dests = [qT, kT, vT]
            for oi in range(3):
                for n in range(NCH):
                    ps = ps_s.tile([128, NCHUNK], F32, tag="s", name=f"qkv{oi}_{n}")
                    for ci in range(2):
                        nc.tensor.matmul(
                            ps,
                            lhsT=wqkvT_t[ci][:, 128 * oi:128 * (oi + 1)],
                            rhs=h_t[ci][:, NCHUNK * n:NCHUNK * (n + 1)],
                            start=(ci == 0), stop=(ci == 1),
                        )
                    nc.vector.tensor_scalar(
                        out=dests[oi][:, NCHUNK * n:NCHUNK * (n + 1)],
                        in0=ps, scalar1=qkvb_t[oi], scalar2=None, op0=ALU.add,
                    )

            # v into [pixel, d] layout: PE transpose of vT 128x128 tiles.
            # Per head: v_h[:, g, pair, :] = [v(64) | ones]; DoubleRow lhsT
            # slices [128, 2, 65] are contiguous; sums land at out row 64.
            v_all = pp.tile([128, MCH, 130], BF16, tag="v_all", name="v_all")
            nc.gpsimd.memset(v_all[:, :, 64:65], 1.0)
            nc.gpsimd.memset(v_all[:, :, 129:130], 1.0)
            for j in range(MCH):
                pst = ps_s.tile([128, 128], BF16, tag="s", name=f"vtr{j}")
                nc.tensor.transpose(pst, vT[:, 128 * j:128 * (j + 1)], ident)
                nc.vector.tensor_copy(out=v_all[:, j, 0:64], in_=pst[:, 0:64])
                nc.vector.tensor_copy(out=v_all[:, j, 65:129], in_=pst[:, 64:128])


            # ================= Phase 3: attention + proj =================
            # Software-pipelined: chunk n's normalization+proj tail is emitted
            # in the middle of chunk n+1's score/attnv group loop so the PE
            # never waits on the DVE/GPSIMD reciprocal chain.
            def emit_tail(po, n):
                nsl = slice(NCHUNK * n, NCHUNK * (n + 1))
                rbh = []
                for hh in range(2):
                    rr = tp.tile([1, NCHUNK], F32, tag=f"rr{hh}", name=f"rr{hh}_{n}")
                    nc.vector.tensor_copy(out=rr, in_=po[hh][64:65, :])
                    nc.vector.reciprocal_approx_fast(out=rr, in_=rr)
                    rb = tp.tile([64, NCHUNK], F32, tag=f"rb{hh}", name=f"rb{hh}_{n}")
                    nc.gpsimd.partition_broadcast(rb, rr, channels=64)
                    rbh.append(rb)
                onrm = tp.tile([128, NCHUNK], F32R, tag="onrm", name=f"onrm_{n}")
                nc.vector.tensor_mul(out=onrm[0:64, :], in0=po[0][0:64, :], in1=rbh[0])
                nc.vector.tensor_mul(out=onrm[64:128, :], in0=po[1][0:64, :], in1=rbh[1])
                for ci in range(2):
                    ppj = ps_s.tile([128, NCHUNK], F32, tag="s", name=f"proj{ci}_{n}")
                    nc.tensor.matmul(ppj, lhsT=pwT_t[ci], rhs=onrm, start=True, stop=True)
                    osb = tp.tile([128, NCHUNK], F32, tag="osb", name=f"osb{ci}_{n}")
                    nc.vector.tensor_copy(out=osb, in_=ppj)
                    nc.sync.dma_start(out=out_d[128 * ci:128 * (ci + 1), nsl], in_=osb)

            pending = None
            for n in range(NCH):
                nsl = slice(NCHUNK * n, NCHUNK * (n + 1))
                po = []
                for hh in range(2):
                    poh = ps_o.tile([65, NCHUNK], F32, tag=f"o{hh}", name=f"po{hh}_{n}")
                    dsl = slice(64 * hh, 64 * (hh + 1))
                    for g in range(MCH // EXPG):
                        pss = ps_s.tile([128, EXPG, NCHUNK], F32, tag="s", name=f"s{n}_{hh}_{g}")
                        for u in range(EXPG):
                            j = EXPG * g + u
                            nc.tensor.matmul(
                                pss[:, u, :],
                                lhsT=kT[dsl, 128 * j:128 * (j + 1)],
                                rhs=qT[dsl, nsl],
                                start=True, stop=True,
                            )
                        at = ap_pool.tile([128, EXPG, NCHUNK], BF16, tag="a", name=f"a{n}_{hh}_{g}")
                        nc.scalar.activation(out=at, in_=pss, func=AF.Exp, scale=SCALE)
                        for u in range(EXPG):
                            j = EXPG * g + u
                            lhs = v_all[:, j, 0:65] if hh == 0 else v_all[:, j, 65:130]
                            nc.tensor.matmul(
                                poh, lhsT=lhs, rhs=at[:, u, :],
                                start=(j == 0), stop=(j == MCH - 1),
                            )
                        if pending is not None and hh == 0 and g == 3:
                            emit_tail(*pending)
                            pending = None
                    po.append(poh)
                pending = (po, n)
            emit_tail(*pending)

    nc.compile()
    return nc


_NC_CACHE = None


def _get_nc():
    global _NC_CACHE
    if _NC_CACHE is None:
        _NC_CACHE = build_bass()
    return _NC_CACHE


def _make_in_maps(x, norm_w, norm_b, qkv_w, qkv_b, proj_w):
    # constant index helper tensors
    ch = np.arange(128)
    indf = np.zeros((2, 128, 8), np.float32)
    indb = np.zeros((2, 8, 128), np.float32)
    for i in range(2):
        g = (i * 128 + ch) // 32
        indf[i, ch, g] = 1.0
        indb[i, g, ch] = 1.0
    nw = norm_w.reshape(2, 128, 1).astype(np.float32)
    nb = norm_b.reshape(2, 128, 1).astype(np.float32)

    in_maps = []
    for core in range(8):
        b, hh = core // 2, core % 2
        sl = slice(128 * hh, 128 * (hh + 1))
        w_slice = np.concatenate(
            [qkv_w[sl], qkv_w[256 + 128 * hh:256 + 128 * (hh + 1)],
             qkv_w[512 + 128 * hh:512 + 128 * (hh + 1)]], axis=0,
        )  # [384, 256]
        wqkvT = np.ascontiguousarray(w_slice.T).astype(np.float32)  # [256, 384]
        qkvb = np.stack(
            [qkv_b[sl], qkv_b[256 + 128 * hh:256 + 128 * (hh + 1)],
             qkv_b[512 + 128 * hh:512 + 128 * (hh + 1)]], axis=0,
        ).reshape(3, 128, 1).astype(np.float32)
        pwT = np.ascontiguousarray(proj_w[:, sl].T).astype(np.float32)  # [128, 256]
        in_maps.append({
            "x": np.ascontiguousarray(x[b].reshape(C, N)).astype(np.float32),
            "wqkvT": wqkvT,
            "qkvb": qkvb,
            "pwT": pwT,
            "nw": nw,
            "nb": nb,
            "indf": indf,
            "indb": indb,
        })
    return in_maps


def kernel(x, norm_w, norm_b, qkv_w, qkv_b, proj_w, proj_b, _trace=False, _tmpdir=None):
    x = np.asarray(x, np.float32)
    norm_w = np.asarray(norm_w, np.float32)
    norm_b = np.asarray(norm_b, np.float32)
    qkv_w = np.asarray(qkv_w, np.float32)
    qkv_b = np.asarray(qkv_b, np.float32)
    proj_w = np.asarray(proj_w, np.float32)
    proj_b = np.asarray(proj_b, np.float32)

    nc = _get_nc()
    in_maps = _make_in_maps(x, norm_w, norm_b, qkv_w, qkv_b, proj_w)
    kw = {}
    if _trace:
        kw = dict(trace=True, tmpdir=_tmpdir)
    res = run_bass_kernel_spmd(nc, in_maps, list(range(8)), **kw)

    out = np.empty((B, C, H, W), np.float32)
    bias_res = proj_b[:, None].astype(np.float32)
    for b in range(B):
        acc = (res.results[2 * b]["out_part"] + res.results[2 * b + 1]["out_part"]
               + bias_res + x[b].reshape(C, N))
        out[b] = acc.reshape(C, H, W)
    if _trace:
        return out, res
    return out
